# revision 16
# baseline (speedup 1.0000x reference)
"""Trainium2 Bass kernel for nn_KGEdges: pairwise edge scores.

S[b,i,j] = sum_d w_out[d] * tanh( (x[b,j]@Wh.T + bh)[d] + (x[b,i]@Wc.T)[d] )
           + minus_mask[b,i] + minus_mask[b,j]

Strategy: data-parallel over batch (8 batches -> 8 cores). Per core:
  - PE: head/child projections (f32, k=1024 accumulated in PSUM)
  - DVE: broadcast-add  sum[d, i] = childb[d, i] + head[d, j]   (fp16, 4x mode)
  - ACT: tanh on large (128, 4096) tiles (the throughput floor)
  - PE:  w_out reduction, one (128x128 stationary, n=1) matmul per
         (j, i_block, d_half), accumulating S[i, j] columns in PSUM.

All fp32 inputs are host-packed into ONE DRAM tensor so they arrive in a
single DMA: the PE's Matmult/LDWEIGHTS encoding only supports one sync
wait, so each DMA-queue semaphore is absorbed into the PE vector clock
by a tiny dummy matmul before any real matmul consumes the data.
"""

import os
import sys

for _p in ("/opt/trn_rl_repo", "/opt/pypackages"):
    if _p not in sys.path and os.path.isdir(_p):
        sys.path.insert(0, _p)

import numpy as np

from concourse import bass, tile
import concourse.mybir as mybir
from concourse.bass_utils import run_bass_kernel_spmd

BS, SL, ENC, ED = 8, 256, 1024, 256
P = 128           # partitions
KO = ENC // P     # k-chunks for projections
DH = ED // P      # d halves
CH = 32           # j columns per main-loop chunk
NCH = SL // CH

# bf16 projection operands, packed per partition row: [x | Wh | Wc]
F_SEC = KO * SL               # one tensor section (ko-major)
F_PRJ = 3 * F_SEC
# f32 tail param: [mj (SL) | bh (DH) | mi (2) | z (1)]
T_MJ = 0
T_BH = SL
T_MI = T_BH + DH
T_Z = T_MI + 2
T_TOT = T_Z + 1

F32 = mybir.dt.float32
F16 = mybir.dt.float16
BF16 = mybir.dt.bfloat16
AF = mybir.ActivationFunctionType

_CACHE: dict = {}

_ENGINE_SEM_PREFIXES = ("Activation", "DVE", "PE", "Pool", "SP", "DMAHW", "DMASW")


def _strip_self_waits(raw: bytes) -> bytes:
    """Remove self-engine semaphore waits that are provably satisfied by
    in-order execution (walrus encodes at most one sync wait per instr).

    An instruction's own semaphore is the one it increments in on_update.
    A wait on that semaphore for a value <= the cumulative increments of
    preceding instructions is always satisfied when the instruction
    reaches the head of its (strictly in-order) engine queue.
    """
    import json

    m = json.loads(raw)
    for fn in m["functions"]:
        seen: dict = {}  # sem id -> cumulative increments in program order
        for blk in fn["blocks"]:
            for ins in blk["instructions"]:
                si = ins.get("sync_info") or {}
                upd = si.get("on_update") or []
                own = {
                    u["id"]
                    for u in upd
                    if u.get("sync_type") == "semaphore"
                    and str(u.get("ant_name", "")).startswith(_ENGINE_SEM_PREFIXES)
                }
                ow = si.get("on_wait") or []
                if len(ow) >= 2:
                    kept = []
                    for w in ow:
                        if (
                            w.get("sync_type") == "semaphore"
                            and w["id"] in own
                            and w.get("wait_mode") == "sem-ge-imm"
                            and w.get("wait_value", 1 << 30)
                            <= seen.get(w["id"], 0)
                        ):
                            continue  # provably satisfied self-wait
                        kept.append(w)
                    si["on_wait"] = kept
                for u in upd:
                    if u.get("sync_type") == "semaphore" and u.get(
                        "update_mode"
                    ) in ("sem-inc", "sem-add-imm"):
                        seen[u["id"]] = seen.get(u["id"], 0) + u.get(
                            "update_value", 1
                        )
        # split residual multi-waits on operand-free sync instructions
        # (e.g. the kernel-tail Drain) into clones with one wait each
        nid = [1 << 20]
        for blk in fn["blocks"]:
            out_insts = []
            for ins in blk["instructions"]:
                si = ins.get("sync_info") or {}
                ow = si.get("on_wait") or []
                if len(ow) >= 2 and not ins.get("ins") and not ins.get("outs"):
                    for w in ow[:-1]:
                        clone = json.loads(json.dumps(ins))
                        clone["sync_info"]["on_wait"] = [w]
                        clone["sync_info"]["on_update"] = []
                        clone["name"] = f"I-{nid[0]}"
                        nid[0] += 1
                        out_insts.append(clone)
                    si["on_wait"] = [ow[-1]]
                out_insts.append(ins)
            blk["instructions"] = out_insts
    return json.dumps(m).encode()


def _build():
    nc = bass.Bass()

    inpb = nc.declare_dram_parameter("inpb", [P, F_PRJ], BF16, isOutput=False)
    tailp = nc.declare_dram_parameter("tailp", [P, T_TOT], F32, isOutput=False)
    w16 = nc.declare_dram_parameter("w16", [P, DH], F16, isOutput=False)
    S_out = nc.declare_dram_parameter("S", [SL, SL], F32, isOutput=True)

    with tile.TileContext(nc) as tc:
        with (
            tc.tile_pool(name="const", bufs=1) as cpool,
            tc.tile_pool(name="work", bufs=3) as wpool,
            tc.tile_pool(name="pproj", bufs=2, space=bass.MemorySpace.PSUM) as pproj,
            tc.tile_pool(name="pacc", bufs=1, space=bass.MemorySpace.PSUM) as pacc,
            tc.tile_pool(name="pjunk", bufs=1, space=bass.MemorySpace.PSUM) as pjunk,
        ):
            NQ = 1  # one DMA per section (x / Wh / Wc)
            QW = F_SEC // NQ
            # tiny tail first: it gates the ACT/DVE absorbers
            tail_sb = cpool.tile([P, T_TOT], F32, tag="tail")
            nc.sync.dma_start(out=tail_sb[:, :], in_=tailp[:, :])
            inp_sb = cpool.tile([P, F_PRJ], BF16, tag="inp")
            for sec in range(3):
                for q in range(NQ):
                    lo = sec * F_SEC + q * QW
                    nc.sync.dma_start(
                        out=inp_sb[:, lo : lo + QW], in_=inpb[:, lo : lo + QW]
                    )
            w_sb = cpool.tile([P, DH], F16, tag="w")
            nc.sync.dma_start(out=w_sb[:, :], in_=w16[:, :])

            def proj_sl(t, ko, lo, hi):
                base = t * F_SEC + ko * SL
                return inp_sb[:, base + lo : base + hi]

            mj_sb = tail_sb[:, T_MJ : T_MJ + SL]
            bh_sb = tail_sb[:, T_BH : T_BH + DH]
            mi_sb = tail_sb[:, T_MI : T_MI + 2]
            zero_b = tail_sb[:, T_Z : T_Z + 1]

            # absorb each DMA semaphore into each consuming engine's vector
            # clock (walrus encodes at most ONE sync wait per instruction).
            junk = pjunk.tile([1, 16], F32, tag="junk")
            junk_n = [0]

            def absorb(col):
                k = junk_n[0]
                junk_n[0] += 1
                nc.tensor.matmul(
                    junk[:, k : k + 1],
                    inp_sb[:, col : col + 1],
                    inp_sb[:, col : col + 1],
                    start=True,
                    stop=True,
                    skip_group_check=True,
                )

            junk_act = cpool.tile([P, 1], F32, tag="junk_act")
            nc.scalar.copy(junk_act[:, :], tail_sb[:, 0:1])
            junk_dve = cpool.tile([P, 1], F32, tag="junk_dve")
            nc.vector.tensor_copy(junk_dve[:, :], tail_sb[:, 0:1])

            # ---- projections: head_T[d, s] (+bh) and child_T[d, s]
            headb = cpool.tile([P, DH, SL], F32, tag="headb")  # bias source
            childb = cpool.tile([P, DH, SL], F16, tag="childb")  # streamed operand
            for mh in range(DH):
                ps_h = pproj.tile([P, SL], F32, tag="proj")
                for ko in range(KO):
                    if mh == 0 and ko % (KO // NQ) == 0:
                        q = ko // (KO // NQ)
                        absorb(q * QW)          # x quarter q
                        absorb(F_SEC + q * QW)  # Wh quarter q
                    nc.tensor.matmul(
                        ps_h[:, :],
                        proj_sl(1, ko, mh * P, (mh + 1) * P),
                        proj_sl(0, ko, 0, SL),
                        start=(ko == 0),
                        stop=(ko == KO - 1),
                    )
                nc.scalar.activation(
                    headb[:, mh, :], ps_h[:, :], AF.Identity, bias=bh_sb[:, mh : mh + 1]
                )
                if mh == 0:
                    for q in range(NQ):
                        absorb(2 * F_SEC + q * QW)  # Wc quarters
                    nc.tensor.matmul(
                        junk[:, 15:16], w_sb[:, 0:1], w_sb[:, 0:1],
                        start=True, stop=True, skip_group_check=True,
                    )
                ps_c = pproj.tile([P, SL], F32, tag="proj")
                for ko in range(KO):
                    nc.tensor.matmul(
                        ps_c[:, :],
                        proj_sl(2, ko, mh * P, (mh + 1) * P),
                        proj_sl(0, ko, 0, SL),
                        start=(ko == 0),
                        stop=(ko == KO - 1),
                    )
                nc.vector.tensor_copy(childb[:, mh, :], ps_c[:, :])

            # ---- main loop: S[i, j] accumulated in PSUM, i on partitions.
            # Accumulators are split by j-half so the first half's epilogue
            # and output DMA overlap the second half of the loop.
            acc00 = pacc.tile([P, P], F32, tag="acc00")
            acc01 = pacc.tile([P, P], F32, tag="acc01")
            acc10 = pacc.tile([P, P], F32, tag="acc10")
            acc11 = pacc.tile([P, P], F32, tag="acc11")
            acc = [[acc00, acc01], [acc10, acc11]]

            def epilogue(jh):
                for ib in range(2):
                    s_t = cpool.tile([P, P], F32, tag=f"sout{ib}{jh}")
                    nc.vector.tensor_scalar_add(
                        s_t[:, :], acc[ib][jh][:, :], mi_sb[:, ib : ib + 1]
                    )
                    nc.vector.tensor_add(
                        s_t[:, :], s_t[:, :], mj_sb[:, jh * P : (jh + 1) * P]
                    )
                    nc.sync.dma_start(
                        out=S_out[ib * P : (ib + 1) * P, jh * P : (jh + 1) * P],
                        in_=s_t[:, :],
                    )

            for jc in range(NCH):
                acts = []
                for h in range(DH):
                    sum_t = wpool.tile([P, CH, SL], F16, tag="sum")
                    nc.vector.tensor_copy(sum_t[:, 0, 0:1], sum_t[:, 0, 0:1])
                    for jj in range(CH):
                        j = jc * CH + jj
                        nc.vector.tensor_scalar_add(
                            sum_t[:, jj, :], childb[:, h, :], headb[:, h, j : j + 1]
                        )
                    act_t = wpool.tile([P, CH, SL], F16, tag="act")
                    # slot-claim: first write carries the slot-recycle (PE)
                    # wait so the tanh itself only waits on DVE
                    nc.scalar.copy(act_t[:, 0, 0:1], act_t[:, 0, 0:1])
                    nc.scalar.activation(
                        act_t[:, :, :], sum_t[:, :, :], AF.Tanh, bias=zero_b
                    )
                    acts.append(act_t)
                for jj in range(CH):
                    j = jc * CH + jj
                    jh, jr = j // P, j % P
                    for ib in range(2):
                        for h in range(DH):
                            nc.tensor.matmul(
                                acc[ib][jh][:, jr : jr + 1],
                                acts[h][:, jj, ib * P : (ib + 1) * P],
                                w_sb[:, h : h + 1],
                                start=(h == 0),
                                stop=(h == DH - 1),
                            )
                if jc == NCH // 2 - 1:
                    epilogue(0)
            epilogue(1)

    _orig = nc.to_json_bytes
    nc.to_json_bytes = lambda: _strip_self_waits(_orig())
    return nc


def _prep_in_maps(inputs):
    x = np.ascontiguousarray(np.asarray(inputs["encoded_text"], dtype=np.float32))
    mask = np.asarray(inputs["mask"])
    Wh = np.asarray(inputs["Wh"], dtype=np.float32)
    bh = np.asarray(inputs["bh"], dtype=np.float32)
    Wc = np.asarray(inputs["Wc"], dtype=np.float32)
    w_out = np.asarray(inputs["w_out"], dtype=np.float32)

    import ml_dtypes

    bf16 = ml_dtypes.bfloat16
    # partition-major (P, KO*SL) sections in bf16
    WhS = np.ascontiguousarray(
        Wh.T.reshape(KO, P, SL).transpose(1, 0, 2).reshape(P, F_SEC)
    ).astype(bf16)
    WcS = np.ascontiguousarray(
        Wc.T.reshape(KO, P, SL).transpose(1, 0, 2).reshape(P, F_SEC)
    ).astype(bf16)
    mm = ((1.0 - mask.astype(np.float32)) * -1.0e8).astype(np.float32)  # (BS, SL)
    w16 = np.ascontiguousarray(w_out.astype(np.float16).reshape(DH, P).T)  # (P, DH)

    in_maps = []
    for b in range(BS):
        xS = np.ascontiguousarray(
            x[b].T.reshape(KO, P, SL).transpose(1, 0, 2).reshape(P, F_SEC)
        ).astype(bf16)
        packed = np.empty((P, F_PRJ), dtype=bf16)
        packed[:, 0:F_SEC] = xS
        packed[:, F_SEC : 2 * F_SEC] = WhS
        packed[:, 2 * F_SEC : 3 * F_SEC] = WcS
        tailv = np.zeros((P, T_TOT), dtype=np.float32)
        tailv[:, T_MJ : T_MJ + SL] = mm[b][None, :]
        tailv[:, T_BH : T_BH + DH] = bh.reshape(DH, P).T
        tailv[:, T_MI : T_MI + 2] = mm[b].reshape(2, P).T
        tailv[:, T_Z] = 0.0
        in_maps.append(dict(inpb=packed, tailp=tailv, w16=w16))
    return in_maps


def run(inputs, trace=False, **kw):
    """Build (cached), run on 8 cores, return (full_output, BassKernelResults)."""
    if "nc" not in _CACHE:
        _CACHE["nc"] = _build()
    nc = _CACHE["nc"]
    in_maps = _prep_in_maps(inputs)
    res = run_bass_kernel_spmd(nc, in_maps, list(range(BS)), trace=trace, **kw)
    out = np.stack([np.asarray(res.results[b]["S"]) for b in range(BS)], axis=0)
    return out.astype(np.float32, copy=False), res


def kernel(**inputs):
    return run(inputs)[0]
